# revision 34
# baseline (speedup 1.0000x reference)
"""MoE FFN layer (8 experts, top-2) on 8 Trainium2 NeuronCores.

Strategy (expert-parallel, per the sharding hint):
  - Router (gate matmul, softmax, top-2, combine weights, aux loss) runs on
    host in fp32 numpy — it is tiny (~67 MFLOP) next to the FFN.
  - Each core c is assigned expert c. The host gathers the tokens routed to
    each expert, pads to a common capacity C_pad (SPMD: one program, eight
    data sets), and pre-packs tokens + that expert's weights into the exact
    SBUF layouts the kernel wants (bf16, transposed so no on-device
    transposes are needed).
  - On-device per core: hT = wg_e @ x_e.T and uT = wu_e @ x_e.T with I on
    partitions and tokens on the free dim, p = silu(h)*u (ACT + DVE), then
    yT = wd_e.T-partials @ p with H on partitions. The I=4096 contraction is
    split into two halves so the p buffer fits SBUF; the two fp32 partial
    yT halves are summed on host.
  - Host scatters y_e back per token with the combine weights (expert order
    matches the reference accumulation order) and computes the aux loss.
"""

import sys
import types

import numpy as np
import ml_dtypes

sys.path.insert(0, "/root/.axon_site")


def _install_ntff_hook():
    """antenv.axon_hooks is missing on this image; shim it so trace=True
    (BASS_TRACE=1) can produce exec_time_ns. Harmless when tracing is off."""
    try:
        import antenv.axon_hooks  # noqa: F401
        return
    except ImportError:
        pass
    try:
        import trn_agent_boot.trn_boot as tb
        hook = tb._ntff_profile_via_ctypes("/opt/axon/libaxon_pjrt.so")
    except Exception:
        hook = None
    mod = types.ModuleType("antenv.axon_hooks")
    mod.get_axon_ntff_profile_hook = lambda: hook
    mod.set_axon_ntff_profile_hook = lambda h: None
    sys.modules["antenv.axon_hooks"] = mod


_install_ntff_hook()

import concourse.bass as bass  # noqa: E402
import concourse.mybir as mybir  # noqa: E402
from concourse import bacc  # noqa: E402
from concourse import bass_utils  # noqa: E402
from concourse.tile import TileContext  # noqa: E402

E = 8          # experts == cores
TOP_K = 2
H = 1024       # hidden
I = 4096       # intermediate
P = 128        # partitions
KH = H // P    # 8 K-chunks for gate/up contraction
NI = I // P    # 32 I-chunks
NH = H // P    # 8 H-chunks for down-proj output
N_HALF = 2     # split I contraction into halves for SBUF fit
NI_H = NI // N_HALF  # 16 I-chunks per half
BLK = 512      # token block (PE free dim / one PSUM bank)

BF16 = ml_dtypes.bfloat16

# Cache of compiled programs keyed by C_pad so repeat calls don't recompile.
_PROGRAM_CACHE: dict[int, object] = {}

# Exposed for the test harness: BassKernelResults of the last device run.
LAST_RESULT = None


_SIM_ACT_SWAP = False  # simtest only: CoreSim has no Silu; swap to Sigmoid


def _act_fn():
    if _SIM_ACT_SWAP:
        return mybir.ActivationFunctionType.Sigmoid
    return mybir.ActivationFunctionType.Silu


def _token_blocks(c_pad):
    """c_pad is a multiple of BLK; every block is a full-width 512 so each
    matmul streams the maximum free dim."""
    assert c_pad % BLK == 0
    return [(t, BLK) for t in range(0, c_pad, BLK)]


def _build_program(c_pad):
    """One SPMD program, run on all 8 cores with per-core (expert) data."""
    dt = mybir.dt
    nc = bacc.Bacc("TRN2", target_bir_lowering=False, debug=False)

    blocks = _token_blocks(c_pad)
    nblk = len(blocks)

    xk = nc.dram_tensor(
        "xk", [KH, nblk, P, BLK], dt.bfloat16, kind="ExternalInput"
    ).ap()
    wgp = nc.dram_tensor("wgp", [NI, P, H], dt.bfloat16, kind="ExternalInput").ap()
    wup = nc.dram_tensor("wup", [NI, P, H], dt.bfloat16, kind="ExternalInput").ap()
    wdp = nc.dram_tensor(
        "wdp", [N_HALF, NH, P, NI_H * P], dt.bfloat16, kind="ExternalInput"
    ).ap()
    yp = nc.dram_tensor(
        "yp", [N_HALF, NH, nblk, P, BLK], dt.float32, kind="ExternalOutput"
    ).ap()

    with TileContext(nc) as tc:
        with (
            tc.tile_pool(name="xpool", bufs=1) as xpool,
            tc.tile_pool(name="ppool", bufs=1) as ppool,
            tc.tile_pool(name="wpool", bufs=2) as wpool,
            tc.tile_pool(name="wdpool", bufs=2) as wdpool,
            tc.tile_pool(name="gpool", bufs=4) as gpool,
            tc.tile_pool(name="ypool", bufs=4) as ypool,
            tc.tile_pool(name="pspool", bufs=3, space="PSUM") as pspool,
            tc.tile_pool(name="psypool", bufs=2, space="PSUM") as psypool,
        ):
            # Warm the PE clock (HAM un-throttles after ~3.4us of sustained
            # activity) during the DMA lead-in with throwaway matmuls on a
            # zeroed scratch tile, so the real matmuls all run at full rate.
            warm_sb = xpool.tile([P, BLK], dt.bfloat16, tag="warm")
            nc.vector.memset(warm_sb[:], 0.0)
            warm_ps = psypool.tile([P, BLK], dt.float32, tag="py")
            for _ in range(14):
                nc.tensor.matmul(
                    warm_ps[:], warm_sb[:, :P], warm_sb[:], start=True, stop=True
                )

            # First gate/up slivers before the token DMAs so the PE can
            # start as soon as block 0 of the tokens lands.
            wg0 = wpool.tile([P, H], dt.bfloat16, tag="wg")
            nc.sync.dma_start(out=wg0[:], in_=wgp[0])
            wu0 = wpool.tile([P, H], dt.bfloat16, tag="wu")
            nc.sync.dma_start(out=wu0[:], in_=wup[0])

            # Tokens resident for the whole kernel, one tile per (k-chunk,
            # token-block) so dependencies are fine-grained: the first
            # matmul group only waits for block 0, not all of x.
            xs = {}
            for b_i, (b0, bs) in enumerate(blocks):
                for k in range(KH):
                    xt = xpool.tile([P, bs], dt.bfloat16, tag=f"x_{k}_{b_i}")
                    nc.sync.dma_start(out=xt[:], in_=xk[k, b_i])
                    xs[(k, b_i)] = xt

            for s in range(N_HALF):
                # p buffer for this half: p_sb[:, il*c_pad + t]
                p_sb = ppool.tile([P, NI_H * c_pad], dt.bfloat16, tag="p")

                # Phase A: gate/up matmuls + silu*u for the 16 I-chunks.
                for il in range(NI_H):
                    i = s * NI_H + il
                    if i == 0:
                        wg_sb, wu_sb = wg0, wu0
                    else:
                        wg_sb = wpool.tile([P, H], dt.bfloat16, tag="wg")
                        nc.sync.dma_start(out=wg_sb[:], in_=wgp[i])
                        wu_sb = wpool.tile([P, H], dt.bfloat16, tag="wu")
                        nc.sync.dma_start(out=wu_sb[:], in_=wup[i])
                    for b_i, (b0, bs) in enumerate(blocks):
                        ph = pspool.tile([P, BLK], dt.float32, tag="ph")
                        pu = pspool.tile([P, BLK], dt.float32, tag="pu")
                        for k in range(KH):
                            nc.tensor.matmul(
                                ph[:, :bs],
                                wg_sb[:, k * P : (k + 1) * P],
                                xs[(k, b_i)][:],
                                start=(k == 0),
                                stop=(k == KH - 1),
                            )
                        for k in range(KH):
                            nc.tensor.matmul(
                                pu[:, :bs],
                                wu_sb[:, k * P : (k + 1) * P],
                                xs[(k, b_i)][:],
                                start=(k == 0),
                                stop=(k == KH - 1),
                            )
                        g_sb = gpool.tile([P, BLK], dt.bfloat16, tag="g")
                        nc.scalar.activation(
                            g_sb[:, :bs], ph[:, :bs], _act_fn()
                        )
                        nc.vector.tensor_mul(
                            p_sb[:, il * c_pad + b0 : il * c_pad + b0 + bs],
                            g_sb[:, :bs],
                            pu[:, :bs],
                        )

                # Phase B: down-proj partial for this half (host sums the
                # two fp32 partials).
                for h in range(NH):
                    wd_sb = wdpool.tile([P, NI_H * P], dt.bfloat16, tag="wd")
                    nc.sync.dma_start(out=wd_sb[:], in_=wdp[s, h])
                    for b_i, (b0, bs) in enumerate(blocks):
                        py = psypool.tile([P, BLK], dt.float32, tag="py")
                        for kl in range(NI_H):
                            nc.tensor.matmul(
                                py[:, :bs],
                                wd_sb[:, kl * P : (kl + 1) * P],
                                p_sb[:, kl * c_pad + b0 : kl * c_pad + b0 + bs],
                                start=(kl == 0),
                                stop=(kl == NI_H - 1),
                            )
                        y_sb = ypool.tile([P, BLK], dt.float32, tag="y")
                        nc.vector.tensor_copy(y_sb[:, :bs], py[:, :bs])
                        nc.sync.dma_start(out=yp[s, h, b_i], in_=y_sb[:, :bs])

    nc.compile()
    return nc


def _route(xf, gate_w):
    """fp32 router matching the jax reference semantics."""
    logits = xf @ gate_w.T  # [T, E]
    m = logits.max(axis=-1, keepdims=True)
    ex = np.exp(logits - m)
    probs = ex / ex.sum(axis=-1, keepdims=True)
    # top-2, ties -> lower index first (matches jax.lax.top_k)
    order = np.argsort(-probs, axis=-1, kind="stable")
    topk_idx = order[:, :TOP_K].astype(np.int32)
    topk_probs = np.take_along_axis(probs, topk_idx, axis=-1)
    denom = np.clip(topk_probs.sum(axis=-1, keepdims=True), 1e-8, None)
    topk_w = topk_probs / denom
    return probs, topk_idx, topk_w


def _aux_loss(probs, topk_idx, T):
    usage = np.zeros(E, np.float32)
    for k in range(TOP_K):
        usage += np.bincount(topk_idx[:, k], minlength=E).astype(np.float32)
    usage /= max(T * TOP_K, 1)
    importance = probs.mean(axis=0)
    importance = importance / np.clip(importance.sum(), 1e-8, None)
    aux = min(float((usage * importance).sum()) * E, 1.0) * 0.01
    return np.float32(aux)


def kernel(x, gate_w, wg, wu, wd):
    global LAST_RESULT
    x = np.asarray(x, np.float32)
    gate_w = np.asarray(gate_w, np.float32)
    wg = np.asarray(wg, np.float32)
    wu = np.asarray(wu, np.float32)
    wd = np.asarray(wd, np.float32)

    B, S, _ = x.shape
    T = B * S
    xf = x.reshape(T, H)

    probs, topk_idx, topk_w = _route(xf, gate_w)

    # Tokens per expert.
    expert_tokens = []
    counts = np.zeros(E, np.int64)
    for e in range(E):
        mask = (topk_idx == e).any(axis=1)
        idx_e = np.nonzero(mask)[0]
        expert_tokens.append(idx_e)
        counts[e] = idx_e.size
    # Device capacity: capacity factor 1.0 rounded to full 512-token blocks
    # (every matmul gets the max free dim). Tokens beyond an expert's
    # capacity (a fraction of a percent for balanced routing) spill to a
    # host fp32 pass below.
    c_mean = T * TOP_K // E
    c_pad = max(BLK, ((c_mean + BLK - 1) // BLK) * BLK)

    nc = _PROGRAM_CACHE.get(c_pad)
    if nc is None:
        nc = _build_program(c_pad)
        _PROGRAM_CACHE[c_pad] = nc

    xf_bf = xf.astype(BF16)
    in_maps = []
    for e in range(E):
        idx_e = expert_tokens[e][:c_pad]
        ce = idx_e.size
        # tokens: xk[k, b, p, t'] = x_e[b*BLK+t', k*128+p]
        x_e = np.zeros((c_pad, H), BF16)
        x_e[:ce] = xf_bf[idx_e]
        xk = np.ascontiguousarray(
            x_e.reshape(c_pad // BLK, BLK, KH, P).transpose(2, 0, 3, 1)
        )
        # gate/up: wgp[i, p, kk*128+m] = wg[e, i*128+m, kk*128+p]
        wg_e = wg[e].astype(BF16)
        wu_e = wu[e].astype(BF16)
        wgp = np.ascontiguousarray(
            wg_e.reshape(NI, P, KH, P).transpose(0, 3, 2, 1).reshape(NI, P, H)
        )
        wup = np.ascontiguousarray(
            wu_e.reshape(NI, P, KH, P).transpose(0, 3, 2, 1).reshape(NI, P, H)
        )
        # down: wdp[s, h, p, kl*128+m] = wd[e, h*128+m, (s*16+kl)*128+p]
        wd_e = wd[e].astype(BF16)
        wdp = np.ascontiguousarray(
            wd_e.reshape(NH, P, N_HALF, NI_H, P)
            .transpose(2, 0, 4, 3, 1)
            .reshape(N_HALF, NH, P, NI_H * P)
        )
        in_maps.append({"xk": xk, "wgp": wgp, "wup": wup, "wdp": wdp})

    res = bass_utils.run_bass_kernel_spmd(
        nc, in_maps, core_ids=list(range(E))
    )
    LAST_RESULT = res

    # Unshard: y_e[t, h*128+p] = sum_s yp[s, h, p, t]
    def combine_w(idx, e):
        sel = topk_idx[idx] == e  # [n, TOP_K]
        return (topk_w[idx] * sel).sum(axis=1).astype(np.float32)

    def silu(v):
        return v / (1.0 + np.exp(-v))

    out = np.zeros((T, H), np.float32)
    for e in range(E):
        ype = res.results[e]["yp"]  # [2, NH, nblk, 128, BLK] fp32
        # y_e[b*BLK+t', h*128+p] = sum_s ype[s, h, b, p, t']
        y_t = (ype[0] + ype[1]).transpose(1, 3, 0, 2).reshape(c_pad, H)
        idx_dev = expert_tokens[e][:c_pad]
        out[idx_dev] += combine_w(idx_dev, e)[:, None] * y_t[: idx_dev.size]
        # capacity spill: host fp32 pass for the few overflow tokens
        idx_sp = expert_tokens[e][c_pad:]
        if idx_sp.size:
            x_sp = xf[idx_sp]
            y_sp = (silu(x_sp @ wg[e].T) * (x_sp @ wu[e].T)) @ wd[e].T
            out[idx_sp] += combine_w(idx_sp, e)[:, None] * y_sp

    aux = _aux_loss(probs, topk_idx, T)
    return out.reshape(B, S, H), aux


# revision 35
# speedup vs baseline: 1.1960x; 1.1960x over previous
"""MoE FFN layer (8 experts, top-2) on 8 Trainium2 NeuronCores.

Strategy (expert-parallel, per the sharding hint):
  - Router (gate matmul, softmax, top-2, combine weights, aux loss) runs on
    host in fp32 numpy — it is tiny (~67 MFLOP) next to the FFN.
  - Each core c is assigned expert c. The host gathers the tokens routed to
    each expert, pads to a common capacity C_pad (SPMD: one program, eight
    data sets), and pre-packs tokens + that expert's weights into the exact
    SBUF layouts the kernel wants (bf16, transposed so no on-device
    transposes are needed).
  - On-device per core: hT = wg_e @ x_e.T and uT = wu_e @ x_e.T with I on
    partitions and tokens on the free dim, p = silu(h)*u (ACT + DVE), then
    yT = wd_e.T-partials @ p with H on partitions. The I=4096 contraction is
    split into two halves so the p buffer fits SBUF; the two fp32 partial
    yT halves are summed on host.
  - Host scatters y_e back per token with the combine weights (expert order
    matches the reference accumulation order) and computes the aux loss.
"""

import sys
import types

import numpy as np
import ml_dtypes

sys.path.insert(0, "/root/.axon_site")


def _install_ntff_hook():
    """antenv.axon_hooks is missing on this image; shim it so trace=True
    (BASS_TRACE=1) can produce exec_time_ns. Harmless when tracing is off."""
    try:
        import antenv.axon_hooks  # noqa: F401
        return
    except ImportError:
        pass
    try:
        import trn_agent_boot.trn_boot as tb
        hook = tb._ntff_profile_via_ctypes("/opt/axon/libaxon_pjrt.so")
    except Exception:
        hook = None
    mod = types.ModuleType("antenv.axon_hooks")
    mod.get_axon_ntff_profile_hook = lambda: hook
    mod.set_axon_ntff_profile_hook = lambda h: None
    sys.modules["antenv.axon_hooks"] = mod


_install_ntff_hook()

import concourse.bass as bass  # noqa: E402
import concourse.mybir as mybir  # noqa: E402
from concourse import bacc  # noqa: E402
from concourse import bass_utils  # noqa: E402
from concourse.tile import TileContext  # noqa: E402

E = 8          # experts == cores
TOP_K = 2
H = 1024       # hidden
I = 4096       # intermediate
P = 128        # partitions
KH = H // P    # 8 K-chunks for gate/up contraction
NI = I // P    # 32 I-chunks
NH = H // P    # 8 H-chunks for down-proj output
N_HALF = 2     # split I contraction into halves for SBUF fit
NI_H = NI // N_HALF  # 16 I-chunks per half
BLK = 512      # token block (PE free dim / one PSUM bank)

BF16 = ml_dtypes.bfloat16

# Cache of compiled programs keyed by C_pad so repeat calls don't recompile.
_PROGRAM_CACHE: dict[int, object] = {}

# Exposed for the test harness: BassKernelResults of the last device run.
LAST_RESULT = None


_SIM_ACT_SWAP = False  # simtest only: CoreSim has no Silu; swap to Sigmoid


def _act_fn():
    if _SIM_ACT_SWAP:
        return mybir.ActivationFunctionType.Sigmoid
    return mybir.ActivationFunctionType.Silu


def _token_blocks(c_pad):
    """c_pad is a multiple of BLK; every block is a full-width 512 so each
    matmul streams the maximum free dim."""
    assert c_pad % BLK == 0
    return [(t, BLK) for t in range(0, c_pad, BLK)]


def _build_program(c_pad):
    """One SPMD program, run on all 8 cores with per-core (expert) data."""
    dt = mybir.dt
    nc = bacc.Bacc("TRN2", target_bir_lowering=False, debug=False)

    blocks = _token_blocks(c_pad)
    nblk = len(blocks)

    xk = nc.dram_tensor(
        "xk", [KH, nblk, P, BLK], dt.bfloat16, kind="ExternalInput"
    ).ap()
    wgp = nc.dram_tensor("wgp", [NI, P, H], dt.bfloat16, kind="ExternalInput").ap()
    wup = nc.dram_tensor("wup", [NI, P, H], dt.bfloat16, kind="ExternalInput").ap()
    wdp = nc.dram_tensor(
        "wdp", [N_HALF, NH, P, NI_H * P], dt.bfloat16, kind="ExternalInput"
    ).ap()
    yp = nc.dram_tensor(
        "yp", [N_HALF, NH, nblk, P, BLK], dt.float32, kind="ExternalOutput"
    ).ap()

    with TileContext(nc) as tc:
        with (
            tc.tile_pool(name="xpool", bufs=1) as xpool,
            tc.tile_pool(name="ppool", bufs=1) as ppool,
            tc.tile_pool(name="wpool", bufs=2) as wpool,
            tc.tile_pool(name="wdpool", bufs=2) as wdpool,
            tc.tile_pool(name="gpool", bufs=4) as gpool,
            tc.tile_pool(name="ypool", bufs=4) as ypool,
            tc.tile_pool(name="pspool", bufs=3, space="PSUM") as pspool,
            tc.tile_pool(name="psypool", bufs=2, space="PSUM") as psypool,
        ):
            # Warm the PE clock (HAM un-throttles after ~3.4us of sustained
            # activity) during the DMA lead-in with throwaway matmuls on a
            # zeroed scratch tile, so the real matmuls all run at full rate.
            warm_sb = xpool.tile([P, BLK], dt.bfloat16, tag="warm")
            nc.vector.memset(warm_sb[:], 0.0)
            warm_ps = psypool.tile([P, BLK], dt.float32, tag="py")
            for _ in range(14):
                nc.tensor.matmul(
                    warm_ps[:], warm_sb[:, :P], warm_sb[:], start=True, stop=True
                )

            # First gate/up slivers before the token DMAs so the PE can
            # start as soon as block 0 of the tokens lands.
            wg0 = wpool.tile([P, H], dt.bfloat16, tag="wg")
            nc.sync.dma_start(out=wg0[:], in_=wgp[0])
            wu0 = wpool.tile([P, H], dt.bfloat16, tag="wu")
            nc.sync.dma_start(out=wu0[:], in_=wup[0])

            # Tokens resident for the whole kernel, one tile per (k-chunk,
            # token-block) so dependencies are fine-grained: the first
            # matmul group only waits for block 0, not all of x.
            xs = {}
            for b_i, (b0, bs) in enumerate(blocks):
                for k in range(KH):
                    xt = xpool.tile([P, bs], dt.bfloat16, tag=f"x_{k}_{b_i}")
                    nc.sync.dma_start(out=xt[:], in_=xk[k, b_i])
                    xs[(k, b_i)] = xt

            for s in range(N_HALF):
                # p buffer for this half: p_sb[:, il*c_pad + t]
                p_sb = ppool.tile([P, NI_H * c_pad], dt.bfloat16, tag="p")

                # Phase A: gate/up matmuls + silu*u for the 16 I-chunks.
                for il in range(NI_H):
                    i = s * NI_H + il
                    if i == 0:
                        wg_sb, wu_sb = wg0, wu0
                    else:
                        wg_sb = wpool.tile([P, H], dt.bfloat16, tag="wg")
                        nc.sync.dma_start(out=wg_sb[:], in_=wgp[i])
                        wu_sb = wpool.tile([P, H], dt.bfloat16, tag="wu")
                        nc.sync.dma_start(out=wu_sb[:], in_=wup[i])
                    for b_i, (b0, bs) in enumerate(blocks):
                        ph = pspool.tile([P, BLK], dt.float32, tag="ph")
                        pu = pspool.tile([P, BLK], dt.float32, tag="pu")
                        for k in range(KH):
                            nc.tensor.matmul(
                                ph[:, :bs],
                                wg_sb[:, k * P : (k + 1) * P],
                                xs[(k, b_i)][:],
                                start=(k == 0),
                                stop=(k == KH - 1),
                            )
                        for k in range(KH):
                            nc.tensor.matmul(
                                pu[:, :bs],
                                wu_sb[:, k * P : (k + 1) * P],
                                xs[(k, b_i)][:],
                                start=(k == 0),
                                stop=(k == KH - 1),
                            )
                        g_sb = gpool.tile([P, BLK], dt.bfloat16, tag="g")
                        nc.scalar.activation(
                            g_sb[:, :bs], ph[:, :bs], _act_fn()
                        )
                        nc.vector.tensor_mul(
                            p_sb[:, il * c_pad + b0 : il * c_pad + b0 + bs],
                            g_sb[:, :bs],
                            pu[:, :bs],
                        )

                # Phase B: down-proj partial for this half (host sums the
                # two fp32 partials).
                for h in range(NH):
                    wd_sb = wdpool.tile([P, NI_H * P], dt.bfloat16, tag="wd")
                    nc.sync.dma_start(out=wd_sb[:], in_=wdp[s, h])
                    for b_i, (b0, bs) in enumerate(blocks):
                        py = psypool.tile([P, BLK], dt.float32, tag="py")
                        for kl in range(NI_H):
                            nc.tensor.matmul(
                                py[:, :bs],
                                wd_sb[:, kl * P : (kl + 1) * P],
                                p_sb[:, kl * c_pad + b0 : kl * c_pad + b0 + bs],
                                start=(kl == 0),
                                stop=(kl == NI_H - 1),
                            )
                        y_sb = ypool.tile([P, BLK], dt.float32, tag="y")
                        nc.vector.tensor_copy(y_sb[:, :bs], py[:, :bs])
                        nc.sync.dma_start(out=yp[s, h, b_i], in_=y_sb[:, :bs])

    nc.compile()
    return nc


def _route(xf, gate_w):
    """fp32 router matching the jax reference semantics."""
    logits = xf @ gate_w.T  # [T, E]
    m = logits.max(axis=-1, keepdims=True)
    ex = np.exp(logits - m)
    probs = ex / ex.sum(axis=-1, keepdims=True)
    # top-2, ties -> lower index first (matches jax.lax.top_k)
    order = np.argsort(-probs, axis=-1, kind="stable")
    topk_idx = order[:, :TOP_K].astype(np.int32)
    topk_probs = np.take_along_axis(probs, topk_idx, axis=-1)
    denom = np.clip(topk_probs.sum(axis=-1, keepdims=True), 1e-8, None)
    topk_w = topk_probs / denom
    return probs, topk_idx, topk_w


def _aux_loss(probs, topk_idx, T):
    usage = np.zeros(E, np.float32)
    for k in range(TOP_K):
        usage += np.bincount(topk_idx[:, k], minlength=E).astype(np.float32)
    usage /= max(T * TOP_K, 1)
    importance = probs.mean(axis=0)
    importance = importance / np.clip(importance.sum(), 1e-8, None)
    aux = min(float((usage * importance).sum()) * E, 1.0) * 0.01
    return np.float32(aux)


def kernel(x, gate_w, wg, wu, wd):
    global LAST_RESULT
    x = np.asarray(x, np.float32)
    gate_w = np.asarray(gate_w, np.float32)
    wg = np.asarray(wg, np.float32)
    wu = np.asarray(wu, np.float32)
    wd = np.asarray(wd, np.float32)

    B, S, _ = x.shape
    T = B * S
    xf = x.reshape(T, H)

    probs, topk_idx, topk_w = _route(xf, gate_w)

    # Tokens per expert.
    expert_tokens = []
    counts = np.zeros(E, np.int64)
    for e in range(E):
        mask = (topk_idx == e).any(axis=1)
        idx_e = np.nonzero(mask)[0]
        expert_tokens.append(idx_e)
        counts[e] = idx_e.size
    # Device capacity: capacity factor 1.0 rounded to full 512-token blocks
    # (every matmul gets the max free dim). Tokens beyond an expert's
    # capacity (a fraction of a percent for balanced routing) spill to a
    # host fp32 pass below.
    c_mean = T * TOP_K // E
    c_pad = max(BLK, ((c_mean + BLK - 1) // BLK) * BLK)

    nc = _PROGRAM_CACHE.get(c_pad)
    if nc is None:
        nc = _build_program(c_pad)
        _PROGRAM_CACHE[c_pad] = nc

    xf_bf = xf.astype(BF16)
    in_maps = []
    for e in range(E):
        idx_e = expert_tokens[e][:c_pad]
        ce = idx_e.size
        # tokens: xk[k, b, p, t'] = x_e[b*BLK+t', k*128+p]
        x_e = np.zeros((c_pad, H), BF16)
        x_e[:ce] = xf_bf[idx_e]
        xk = np.ascontiguousarray(
            x_e.reshape(c_pad // BLK, BLK, KH, P).transpose(2, 0, 3, 1)
        )
        # gate/up: wgp[i, p, kk*128+m] = wg[e, i*128+m, kk*128+p]
        wg_e = wg[e].astype(BF16)
        wu_e = wu[e].astype(BF16)
        wgp = np.ascontiguousarray(
            wg_e.reshape(NI, P, KH, P).transpose(0, 3, 2, 1).reshape(NI, P, H)
        )
        wup = np.ascontiguousarray(
            wu_e.reshape(NI, P, KH, P).transpose(0, 3, 2, 1).reshape(NI, P, H)
        )
        # down: wdp[s, h, p, kl*128+m] = wd[e, h*128+m, (s*16+kl)*128+p]
        wd_e = wd[e].astype(BF16)
        wdp = np.ascontiguousarray(
            wd_e.reshape(NH, P, N_HALF, NI_H, P)
            .transpose(2, 0, 4, 3, 1)
            .reshape(N_HALF, NH, P, NI_H * P)
        )
        in_maps.append({"xk": xk, "wgp": wgp, "wup": wup, "wdp": wdp})

    res = None
    last_err = None
    for _attempt in range(2):
        try:
            res = bass_utils.run_bass_kernel_spmd(
                nc, in_maps, core_ids=list(range(E))
            )
            break
        except Exception as err:  # rare transient NRT device errors
            last_err = err
    if res is None:
        raise last_err
    LAST_RESULT = res

    # Unshard: y_e[t, h*128+p] = sum_s yp[s, h, p, t]
    def combine_w(idx, e):
        sel = topk_idx[idx] == e  # [n, TOP_K]
        return (topk_w[idx] * sel).sum(axis=1).astype(np.float32)

    def silu(v):
        return v / (1.0 + np.exp(-v))

    out = np.zeros((T, H), np.float32)
    for e in range(E):
        ype = res.results[e]["yp"]  # [2, NH, nblk, 128, BLK] fp32
        # y_e[b*BLK+t', h*128+p] = sum_s ype[s, h, b, p, t']
        y_t = (ype[0] + ype[1]).transpose(1, 3, 0, 2).reshape(c_pad, H)
        idx_dev = expert_tokens[e][:c_pad]
        out[idx_dev] += combine_w(idx_dev, e)[:, None] * y_t[: idx_dev.size]
        # capacity spill: host fp32 pass for the few overflow tokens
        idx_sp = expert_tokens[e][c_pad:]
        if idx_sp.size:
            x_sp = xf[idx_sp]
            y_sp = (silu(x_sp @ wg[e].T) * (x_sp @ wu[e].T)) @ wd[e].T
            out[idx_sp] += combine_w(idx_sp, e)[:, None] * y_sp

    aux = _aux_loss(probs, topk_idx, T)
    return out.reshape(B, S, H), aux


# revision 37
# speedup vs baseline: 1.1990x; 1.0025x over previous
"""MoE FFN layer (8 experts, top-2) on 8 Trainium2 NeuronCores.

Strategy (expert-parallel, per the sharding hint):
  - Router (gate matmul, softmax, top-2, combine weights, aux loss) runs on
    host in fp32 numpy — it is tiny (~67 MFLOP) next to the FFN.
  - Each core c is assigned expert c. The host gathers the tokens routed to
    each expert, pads to a common capacity C_pad (SPMD: one program, eight
    data sets), and pre-packs tokens + that expert's weights into the exact
    SBUF layouts the kernel wants (bf16, transposed so no on-device
    transposes are needed).
  - On-device per core: hT = wg_e @ x_e.T and uT = wu_e @ x_e.T with I on
    partitions and tokens on the free dim, p = silu(h)*u (ACT + DVE), then
    yT = wd_e.T-partials @ p with H on partitions. The I=4096 contraction is
    split into two halves so the p buffer fits SBUF; the two fp32 partial
    yT halves are summed on host.
  - Host scatters y_e back per token with the combine weights (expert order
    matches the reference accumulation order) and computes the aux loss.
"""

import sys
import types

import numpy as np
import ml_dtypes

sys.path.insert(0, "/root/.axon_site")


def _install_ntff_hook():
    """antenv.axon_hooks is missing on this image; shim it so trace=True
    (BASS_TRACE=1) can produce exec_time_ns. Harmless when tracing is off."""
    try:
        import antenv.axon_hooks  # noqa: F401
        return
    except ImportError:
        pass
    try:
        import trn_agent_boot.trn_boot as tb
        hook = tb._ntff_profile_via_ctypes("/opt/axon/libaxon_pjrt.so")
    except Exception:
        hook = None
    mod = types.ModuleType("antenv.axon_hooks")
    mod.get_axon_ntff_profile_hook = lambda: hook
    mod.set_axon_ntff_profile_hook = lambda h: None
    sys.modules["antenv.axon_hooks"] = mod


_install_ntff_hook()

import concourse.mybir as mybir  # noqa: E402
from concourse import bacc  # noqa: E402
from concourse import bass_utils  # noqa: E402
from concourse.tile import TileContext  # noqa: E402

E = 8          # experts == cores
TOP_K = 2
H = 1024       # hidden
I = 4096       # intermediate
P = 128        # partitions
KH = H // P    # 8 K-chunks for gate/up contraction
NI = I // P    # 32 I-chunks
NH = H // P    # 8 H-chunks for down-proj output
N_HALF = 2     # split I contraction into halves for SBUF fit
NI_H = NI // N_HALF  # 16 I-chunks per half
BLK = 512      # token block (PE free dim / one PSUM bank)

BF16 = ml_dtypes.bfloat16

# Cache of compiled programs keyed by C_pad so repeat calls don't recompile.
_PROGRAM_CACHE: dict[int, object] = {}

# Exposed for the test harness: BassKernelResults of the last device run.
LAST_RESULT = None


_SIM_ACT_SWAP = False  # simtest only: CoreSim has no Silu; swap to Sigmoid


def _act_fn():
    if _SIM_ACT_SWAP:
        return mybir.ActivationFunctionType.Sigmoid
    return mybir.ActivationFunctionType.Silu


def _token_blocks(c_pad):
    """c_pad is a multiple of BLK; every block is a full-width 512 so each
    matmul streams the maximum free dim."""
    assert c_pad % BLK == 0
    return [(t, BLK) for t in range(0, c_pad, BLK)]


def _build_program(c_pad):
    """One SPMD program, run on all 8 cores with per-core (expert) data."""
    dt = mybir.dt
    nc = bacc.Bacc("TRN2", target_bir_lowering=False, debug=False)

    blocks = _token_blocks(c_pad)
    nblk = len(blocks)

    xk = nc.dram_tensor(
        "xk", [KH, nblk, P, BLK], dt.bfloat16, kind="ExternalInput"
    ).ap()
    wgp = nc.dram_tensor("wgp", [NI, P, H], dt.bfloat16, kind="ExternalInput").ap()
    wup = nc.dram_tensor("wup", [NI, P, H], dt.bfloat16, kind="ExternalInput").ap()
    wdp = nc.dram_tensor(
        "wdp", [N_HALF, NH, P, NI_H * P], dt.bfloat16, kind="ExternalInput"
    ).ap()
    yp = nc.dram_tensor(
        "yp", [N_HALF, NH, nblk, P, BLK], dt.float32, kind="ExternalOutput"
    ).ap()

    with TileContext(nc) as tc:
        with (
            tc.tile_pool(name="xpool", bufs=1) as xpool,
            tc.tile_pool(name="ppool", bufs=1) as ppool,
            tc.tile_pool(name="wpool", bufs=2) as wpool,
            tc.tile_pool(name="wdpool", bufs=2) as wdpool,
            tc.tile_pool(name="gpool", bufs=4) as gpool,
            tc.tile_pool(name="ypool", bufs=4) as ypool,
            tc.tile_pool(name="pspool", bufs=3, space="PSUM") as pspool,
            tc.tile_pool(name="psypool", bufs=2, space="PSUM") as psypool,
        ):
            # Warm the PE clock (HAM un-throttles after ~3.4us of sustained
            # activity) during the DMA lead-in with throwaway matmuls on a
            # zeroed scratch tile, so the real matmuls all run at full rate.
            warm_sb = xpool.tile([P, BLK], dt.bfloat16, tag="warm")
            nc.vector.memset(warm_sb[:], 0.0)
            warm_ps = psypool.tile([P, BLK], dt.float32, tag="py")
            for _ in range(14):
                nc.tensor.matmul(
                    warm_ps[:], warm_sb[:, :P], warm_sb[:], start=True, stop=True
                )

            # First gate/up slivers before the token DMAs so the PE can
            # start as soon as block 0 of the tokens lands.
            wg0 = wpool.tile([P, H], dt.bfloat16, tag="wg")
            nc.sync.dma_start(out=wg0[:], in_=wgp[0])
            wu0 = wpool.tile([P, H], dt.bfloat16, tag="wu")
            nc.sync.dma_start(out=wu0[:], in_=wup[0])

            # Tokens resident for the whole kernel, one tile per (k-chunk,
            # token-block) so dependencies are fine-grained: the first
            # matmul group only waits for block 0, not all of x.
            xs = {}
            for b_i, (b0, bs) in enumerate(blocks):
                for k in range(KH):
                    xt = xpool.tile([P, bs], dt.bfloat16, tag=f"x_{k}_{b_i}")
                    nc.sync.dma_start(out=xt[:], in_=xk[k, b_i])
                    xs[(k, b_i)] = xt

            for s in range(N_HALF):
                # p buffer for this half: p_sb[:, il*c_pad + t]
                p_sb = ppool.tile([P, NI_H * c_pad], dt.bfloat16, tag="p")

                # Phase A: gate/up matmuls + silu*u for the 16 I-chunks.
                for il in range(NI_H):
                    i = s * NI_H + il
                    if i == 0:
                        wg_sb, wu_sb = wg0, wu0
                    else:
                        wg_sb = wpool.tile([P, H], dt.bfloat16, tag="wg")
                        nc.sync.dma_start(out=wg_sb[:], in_=wgp[i])
                        wu_sb = wpool.tile([P, H], dt.bfloat16, tag="wu")
                        nc.sync.dma_start(out=wu_sb[:], in_=wup[i])
                    for b_i, (b0, bs) in enumerate(blocks):
                        ph = pspool.tile([P, BLK], dt.float32, tag="ph")
                        pu = pspool.tile([P, BLK], dt.float32, tag="pu")
                        for k in range(KH):
                            nc.tensor.matmul(
                                ph[:, :bs],
                                wg_sb[:, k * P : (k + 1) * P],
                                xs[(k, b_i)][:],
                                start=(k == 0),
                                stop=(k == KH - 1),
                            )
                        for k in range(KH):
                            nc.tensor.matmul(
                                pu[:, :bs],
                                wu_sb[:, k * P : (k + 1) * P],
                                xs[(k, b_i)][:],
                                start=(k == 0),
                                stop=(k == KH - 1),
                            )
                        g_sb = gpool.tile([P, BLK], dt.bfloat16, tag="g")
                        nc.scalar.activation(
                            g_sb[:, :bs], ph[:, :bs], _act_fn()
                        )
                        nc.vector.tensor_mul(
                            p_sb[:, il * c_pad + b0 : il * c_pad + b0 + bs],
                            g_sb[:, :bs],
                            pu[:, :bs],
                        )

                # Phase B: down-proj partial for this half (host sums the
                # two fp32 partials).
                for h in range(NH):
                    wd_sb = wdpool.tile([P, NI_H * P], dt.bfloat16, tag="wd")
                    nc.sync.dma_start(out=wd_sb[:], in_=wdp[s, h])
                    for b_i, (b0, bs) in enumerate(blocks):
                        py = psypool.tile([P, BLK], dt.float32, tag="py")
                        for kl in range(NI_H):
                            nc.tensor.matmul(
                                py[:, :bs],
                                wd_sb[:, kl * P : (kl + 1) * P],
                                p_sb[:, kl * c_pad + b0 : kl * c_pad + b0 + bs],
                                start=(kl == 0),
                                stop=(kl == NI_H - 1),
                            )
                        y_sb = ypool.tile([P, BLK], dt.float32, tag="y")
                        nc.vector.tensor_copy(y_sb[:, :bs], py[:, :bs])
                        nc.sync.dma_start(out=yp[s, h, b_i], in_=y_sb[:, :bs])

    nc.compile()
    return nc


def _route(xf, gate_w):
    """fp32 router matching the jax reference semantics."""
    logits = xf @ gate_w.T  # [T, E]
    m = logits.max(axis=-1, keepdims=True)
    ex = np.exp(logits - m)
    probs = ex / ex.sum(axis=-1, keepdims=True)
    # top-2, ties -> lower index first (matches jax.lax.top_k)
    order = np.argsort(-probs, axis=-1, kind="stable")
    topk_idx = order[:, :TOP_K].astype(np.int32)
    topk_probs = np.take_along_axis(probs, topk_idx, axis=-1)
    denom = np.clip(topk_probs.sum(axis=-1, keepdims=True), 1e-8, None)
    topk_w = topk_probs / denom
    return probs, topk_idx, topk_w


def _aux_loss(probs, topk_idx, T):
    usage = np.zeros(E, np.float32)
    for k in range(TOP_K):
        usage += np.bincount(topk_idx[:, k], minlength=E).astype(np.float32)
    usage /= max(T * TOP_K, 1)
    importance = probs.mean(axis=0)
    importance = importance / np.clip(importance.sum(), 1e-8, None)
    aux = min(float((usage * importance).sum()) * E, 1.0) * 0.01
    return np.float32(aux)


def kernel(x, gate_w, wg, wu, wd):
    global LAST_RESULT
    x = np.asarray(x, np.float32)
    gate_w = np.asarray(gate_w, np.float32)
    wg = np.asarray(wg, np.float32)
    wu = np.asarray(wu, np.float32)
    wd = np.asarray(wd, np.float32)

    B, S, _ = x.shape
    T = B * S
    xf = x.reshape(T, H)

    probs, topk_idx, topk_w = _route(xf, gate_w)

    # Tokens per expert.
    expert_tokens = []
    for e in range(E):
        mask = (topk_idx == e).any(axis=1)
        expert_tokens.append(np.nonzero(mask)[0])
    # Device capacity: capacity factor 1.0 rounded to full 512-token blocks
    # (every matmul gets the max free dim). Tokens beyond an expert's
    # capacity (a fraction of a percent for balanced routing) spill to a
    # host fp32 pass below.
    c_mean = T * TOP_K // E
    c_pad = max(BLK, ((c_mean + BLK - 1) // BLK) * BLK)

    nc = _PROGRAM_CACHE.get(c_pad)
    if nc is None:
        nc = _build_program(c_pad)
        _PROGRAM_CACHE[c_pad] = nc

    xf_bf = xf.astype(BF16)
    in_maps = []
    for e in range(E):
        idx_e = expert_tokens[e][:c_pad]
        ce = idx_e.size
        # tokens: xk[k, b, p, t'] = x_e[b*BLK+t', k*128+p]
        x_e = np.zeros((c_pad, H), BF16)
        x_e[:ce] = xf_bf[idx_e]
        xk = np.ascontiguousarray(
            x_e.reshape(c_pad // BLK, BLK, KH, P).transpose(2, 0, 3, 1)
        )
        # gate/up: wgp[i, p, kk*128+m] = wg[e, i*128+m, kk*128+p]
        wg_e = wg[e].astype(BF16)
        wu_e = wu[e].astype(BF16)
        wgp = np.ascontiguousarray(
            wg_e.reshape(NI, P, KH, P).transpose(0, 3, 2, 1).reshape(NI, P, H)
        )
        wup = np.ascontiguousarray(
            wu_e.reshape(NI, P, KH, P).transpose(0, 3, 2, 1).reshape(NI, P, H)
        )
        # down: wdp[s, h, p, kl*128+m] = wd[e, h*128+m, (s*16+kl)*128+p]
        wd_e = wd[e].astype(BF16)
        wdp = np.ascontiguousarray(
            wd_e.reshape(NH, P, N_HALF, NI_H, P)
            .transpose(2, 0, 4, 3, 1)
            .reshape(N_HALF, NH, P, NI_H * P)
        )
        in_maps.append({"xk": xk, "wgp": wgp, "wup": wup, "wdp": wdp})

    res = None
    last_err = None
    for _attempt in range(2):
        try:
            res = bass_utils.run_bass_kernel_spmd(
                nc, in_maps, core_ids=list(range(E))
            )
            break
        except Exception as err:  # rare transient NRT device errors
            last_err = err
    if res is None:
        raise last_err
    LAST_RESULT = res

    # Unshard: y_e[t, h*128+p] = sum_s yp[s, h, p, t]
    def combine_w(idx, e):
        sel = topk_idx[idx] == e  # [n, TOP_K]
        return (topk_w[idx] * sel).sum(axis=1).astype(np.float32)

    def silu(v):
        return v / (1.0 + np.exp(-v))

    out = np.zeros((T, H), np.float32)
    for e in range(E):
        ype = res.results[e]["yp"]  # [2, NH, nblk, 128, BLK] fp32
        # y_e[b*BLK+t', h*128+p] = sum_s ype[s, h, b, p, t']
        y_t = (ype[0] + ype[1]).transpose(1, 3, 0, 2).reshape(c_pad, H)
        idx_dev = expert_tokens[e][:c_pad]
        out[idx_dev] += combine_w(idx_dev, e)[:, None] * y_t[: idx_dev.size]
        # capacity spill: host fp32 pass for the few overflow tokens
        idx_sp = expert_tokens[e][c_pad:]
        if idx_sp.size:
            x_sp = xf[idx_sp]
            y_sp = (silu(x_sp @ wg[e].T) * (x_sp @ wu[e].T)) @ wd[e].T
            out[idx_sp] += combine_w(idx_sp, e)[:, None] * y_sp

    aux = _aux_loss(probs, topk_idx, T)
    return out.reshape(B, S, H), aux


# revision 38
# speedup vs baseline: 1.1991x; 1.0001x over previous
"""MoE FFN layer (8 experts, top-2) on 8 Trainium2 NeuronCores.

Strategy (expert-parallel, per the sharding hint):
  - Router (gate matmul, softmax, top-2, combine weights, aux loss) runs on
    host in fp32 numpy — it is tiny (~67 MFLOP) next to the FFN.
  - Each core c is assigned expert c. The host gathers the tokens routed to
    each expert, pads to a common capacity C_pad (SPMD: one program, eight
    data sets), and pre-packs tokens + that expert's weights into the exact
    SBUF layouts the kernel wants (bf16, transposed so no on-device
    transposes are needed).
  - On-device per core: hT = wg_e @ x_e.T and uT = wu_e @ x_e.T with I on
    partitions and tokens on the free dim, p = silu(h)*u (ACT + DVE), then
    yT = wd_e.T-partials @ p with H on partitions. The I=4096 contraction is
    split into two halves so the p buffer fits SBUF; the two fp32 partial
    yT halves are summed on host.
  - Host scatters y_e back per token with the combine weights (expert order
    matches the reference accumulation order) and computes the aux loss.
"""

import sys
import types

import numpy as np
import ml_dtypes

sys.path.insert(0, "/root/.axon_site")


def _install_ntff_hook():
    """antenv.axon_hooks is missing on this image; shim it so trace=True
    (BASS_TRACE=1) can produce exec_time_ns. Harmless when tracing is off."""
    try:
        import antenv.axon_hooks  # noqa: F401
        return
    except ImportError:
        pass
    try:
        import trn_agent_boot.trn_boot as tb
        hook = tb._ntff_profile_via_ctypes("/opt/axon/libaxon_pjrt.so")
    except Exception:
        hook = None
    mod = types.ModuleType("antenv.axon_hooks")
    mod.get_axon_ntff_profile_hook = lambda: hook
    mod.set_axon_ntff_profile_hook = lambda h: None
    sys.modules["antenv.axon_hooks"] = mod


_install_ntff_hook()

import concourse.mybir as mybir  # noqa: E402
from concourse import bacc  # noqa: E402
from concourse import bass_utils  # noqa: E402
from concourse.tile import TileContext  # noqa: E402

E = 8          # experts == cores
TOP_K = 2
H = 1024       # hidden
I = 4096       # intermediate
P = 128        # partitions
KH = H // P    # 8 K-chunks for gate/up contraction
NI = I // P    # 32 I-chunks
NH = H // P    # 8 H-chunks for down-proj output
N_HALF = 2     # split I contraction into halves for SBUF fit
NI_H = NI // N_HALF  # 16 I-chunks per half
BLK = 512      # token block (PE free dim / one PSUM bank)

BF16 = ml_dtypes.bfloat16

# Cache of compiled programs keyed by C_pad so repeat calls don't recompile.
_PROGRAM_CACHE: dict[int, object] = {}

# Exposed for the test harness: BassKernelResults of the last device run.
LAST_RESULT = None


_SIM_ACT_SWAP = False  # simtest only: CoreSim has no Silu; swap to Sigmoid


def _act_fn():
    if _SIM_ACT_SWAP:
        return mybir.ActivationFunctionType.Sigmoid
    return mybir.ActivationFunctionType.Silu


def _token_blocks(c_pad):
    """c_pad is a multiple of BLK; every block is a full-width 512 so each
    matmul streams the maximum free dim."""
    assert c_pad % BLK == 0
    return [(t, BLK) for t in range(0, c_pad, BLK)]


def _build_program(c_pad):
    """One SPMD program, run on all 8 cores with per-core (expert) data."""
    dt = mybir.dt
    nc = bacc.Bacc("TRN2", target_bir_lowering=False, debug=False)

    blocks = _token_blocks(c_pad)
    nblk = len(blocks)

    xk = nc.dram_tensor(
        "xk", [KH, nblk, P, BLK], dt.bfloat16, kind="ExternalInput"
    ).ap()
    wgp = nc.dram_tensor("wgp", [NI, P, H], dt.bfloat16, kind="ExternalInput").ap()
    wup = nc.dram_tensor("wup", [NI, P, H], dt.bfloat16, kind="ExternalInput").ap()
    wdp = nc.dram_tensor(
        "wdp", [N_HALF, NH, P, NI_H * P], dt.bfloat16, kind="ExternalInput"
    ).ap()
    yp = nc.dram_tensor(
        "yp", [N_HALF, NH, nblk, P, BLK], dt.float32, kind="ExternalOutput"
    ).ap()

    with TileContext(nc) as tc:
        with (
            tc.tile_pool(name="xpool", bufs=1) as xpool,
            tc.tile_pool(name="ppool", bufs=1) as ppool,
            tc.tile_pool(name="wpool", bufs=2) as wpool,
            tc.tile_pool(name="wdpool", bufs=2) as wdpool,
            tc.tile_pool(name="gpool", bufs=4) as gpool,
            tc.tile_pool(name="ypool", bufs=4) as ypool,
            tc.tile_pool(name="pspool", bufs=3, space="PSUM") as pspool,
            tc.tile_pool(name="psypool", bufs=2, space="PSUM") as psypool,
        ):
            # Warm the PE clock (HAM un-throttles after ~3.4us of sustained
            # activity) during the DMA lead-in with throwaway matmuls on a
            # zeroed scratch tile, so the real matmuls all run at full rate.
            warm_sb = xpool.tile([P, BLK], dt.bfloat16, tag="warm")
            nc.vector.memset(warm_sb[:], 0.0)
            warm_ps = psypool.tile([P, BLK], dt.float32, tag="py")
            for _ in range(11):
                nc.tensor.matmul(
                    warm_ps[:], warm_sb[:, :P], warm_sb[:], start=True, stop=True
                )

            # First gate/up slivers before the token DMAs so the PE can
            # start as soon as block 0 of the tokens lands.
            wg0 = wpool.tile([P, H], dt.bfloat16, tag="wg")
            nc.sync.dma_start(out=wg0[:], in_=wgp[0])
            wu0 = wpool.tile([P, H], dt.bfloat16, tag="wu")
            nc.sync.dma_start(out=wu0[:], in_=wup[0])

            # Tokens resident for the whole kernel, one tile per (k-chunk,
            # token-block) so dependencies are fine-grained: the first
            # matmul group only waits for block 0, not all of x.
            xs = {}
            for b_i, (b0, bs) in enumerate(blocks):
                for k in range(KH):
                    xt = xpool.tile([P, bs], dt.bfloat16, tag=f"x_{k}_{b_i}")
                    nc.sync.dma_start(out=xt[:], in_=xk[k, b_i])
                    xs[(k, b_i)] = xt

            for s in range(N_HALF):
                # p buffer for this half: p_sb[:, il*c_pad + t]
                p_sb = ppool.tile([P, NI_H * c_pad], dt.bfloat16, tag="p")

                # Phase A: gate/up matmuls + silu*u for the 16 I-chunks.
                for il in range(NI_H):
                    i = s * NI_H + il
                    if i == 0:
                        wg_sb, wu_sb = wg0, wu0
                    else:
                        wg_sb = wpool.tile([P, H], dt.bfloat16, tag="wg")
                        nc.sync.dma_start(out=wg_sb[:], in_=wgp[i])
                        wu_sb = wpool.tile([P, H], dt.bfloat16, tag="wu")
                        nc.sync.dma_start(out=wu_sb[:], in_=wup[i])
                    for b_i, (b0, bs) in enumerate(blocks):
                        ph = pspool.tile([P, BLK], dt.float32, tag="ph")
                        pu = pspool.tile([P, BLK], dt.float32, tag="pu")
                        for k in range(KH):
                            nc.tensor.matmul(
                                ph[:, :bs],
                                wg_sb[:, k * P : (k + 1) * P],
                                xs[(k, b_i)][:],
                                start=(k == 0),
                                stop=(k == KH - 1),
                            )
                        for k in range(KH):
                            nc.tensor.matmul(
                                pu[:, :bs],
                                wu_sb[:, k * P : (k + 1) * P],
                                xs[(k, b_i)][:],
                                start=(k == 0),
                                stop=(k == KH - 1),
                            )
                        g_sb = gpool.tile([P, BLK], dt.bfloat16, tag="g")
                        nc.scalar.activation(
                            g_sb[:, :bs], ph[:, :bs], _act_fn()
                        )
                        nc.vector.tensor_mul(
                            p_sb[:, il * c_pad + b0 : il * c_pad + b0 + bs],
                            g_sb[:, :bs],
                            pu[:, :bs],
                        )

                # Phase B: down-proj partial for this half (host sums the
                # two fp32 partials).
                for h in range(NH):
                    wd_sb = wdpool.tile([P, NI_H * P], dt.bfloat16, tag="wd")
                    nc.sync.dma_start(out=wd_sb[:], in_=wdp[s, h])
                    for b_i, (b0, bs) in enumerate(blocks):
                        py = psypool.tile([P, BLK], dt.float32, tag="py")
                        for kl in range(NI_H):
                            nc.tensor.matmul(
                                py[:, :bs],
                                wd_sb[:, kl * P : (kl + 1) * P],
                                p_sb[:, kl * c_pad + b0 : kl * c_pad + b0 + bs],
                                start=(kl == 0),
                                stop=(kl == NI_H - 1),
                            )
                        y_sb = ypool.tile([P, BLK], dt.float32, tag="y")
                        nc.vector.tensor_copy(y_sb[:, :bs], py[:, :bs])
                        nc.sync.dma_start(out=yp[s, h, b_i], in_=y_sb[:, :bs])

    nc.compile()
    return nc


def _route(xf, gate_w):
    """fp32 router matching the jax reference semantics."""
    logits = xf @ gate_w.T  # [T, E]
    m = logits.max(axis=-1, keepdims=True)
    ex = np.exp(logits - m)
    probs = ex / ex.sum(axis=-1, keepdims=True)
    # top-2, ties -> lower index first (matches jax.lax.top_k)
    order = np.argsort(-probs, axis=-1, kind="stable")
    topk_idx = order[:, :TOP_K].astype(np.int32)
    topk_probs = np.take_along_axis(probs, topk_idx, axis=-1)
    denom = np.clip(topk_probs.sum(axis=-1, keepdims=True), 1e-8, None)
    topk_w = topk_probs / denom
    return probs, topk_idx, topk_w


def _aux_loss(probs, topk_idx, T):
    usage = np.zeros(E, np.float32)
    for k in range(TOP_K):
        usage += np.bincount(topk_idx[:, k], minlength=E).astype(np.float32)
    usage /= max(T * TOP_K, 1)
    importance = probs.mean(axis=0)
    importance = importance / np.clip(importance.sum(), 1e-8, None)
    aux = min(float((usage * importance).sum()) * E, 1.0) * 0.01
    return np.float32(aux)


def kernel(x, gate_w, wg, wu, wd):
    global LAST_RESULT
    x = np.asarray(x, np.float32)
    gate_w = np.asarray(gate_w, np.float32)
    wg = np.asarray(wg, np.float32)
    wu = np.asarray(wu, np.float32)
    wd = np.asarray(wd, np.float32)

    B, S, _ = x.shape
    T = B * S
    xf = x.reshape(T, H)

    probs, topk_idx, topk_w = _route(xf, gate_w)

    # Tokens per expert.
    expert_tokens = []
    for e in range(E):
        mask = (topk_idx == e).any(axis=1)
        expert_tokens.append(np.nonzero(mask)[0])
    # Device capacity: capacity factor 1.0 rounded to full 512-token blocks
    # (every matmul gets the max free dim). Tokens beyond an expert's
    # capacity (a fraction of a percent for balanced routing) spill to a
    # host fp32 pass below.
    c_mean = T * TOP_K // E
    c_pad = max(BLK, ((c_mean + BLK - 1) // BLK) * BLK)

    nc = _PROGRAM_CACHE.get(c_pad)
    if nc is None:
        nc = _build_program(c_pad)
        _PROGRAM_CACHE[c_pad] = nc

    xf_bf = xf.astype(BF16)
    in_maps = []
    for e in range(E):
        idx_e = expert_tokens[e][:c_pad]
        ce = idx_e.size
        # tokens: xk[k, b, p, t'] = x_e[b*BLK+t', k*128+p]
        x_e = np.zeros((c_pad, H), BF16)
        x_e[:ce] = xf_bf[idx_e]
        xk = np.ascontiguousarray(
            x_e.reshape(c_pad // BLK, BLK, KH, P).transpose(2, 0, 3, 1)
        )
        # gate/up: wgp[i, p, kk*128+m] = wg[e, i*128+m, kk*128+p]
        wg_e = wg[e].astype(BF16)
        wu_e = wu[e].astype(BF16)
        wgp = np.ascontiguousarray(
            wg_e.reshape(NI, P, KH, P).transpose(0, 3, 2, 1).reshape(NI, P, H)
        )
        wup = np.ascontiguousarray(
            wu_e.reshape(NI, P, KH, P).transpose(0, 3, 2, 1).reshape(NI, P, H)
        )
        # down: wdp[s, h, p, kl*128+m] = wd[e, h*128+m, (s*16+kl)*128+p]
        wd_e = wd[e].astype(BF16)
        wdp = np.ascontiguousarray(
            wd_e.reshape(NH, P, N_HALF, NI_H, P)
            .transpose(2, 0, 4, 3, 1)
            .reshape(N_HALF, NH, P, NI_H * P)
        )
        in_maps.append({"xk": xk, "wgp": wgp, "wup": wup, "wdp": wdp})

    res = None
    last_err = None
    for _attempt in range(2):
        try:
            res = bass_utils.run_bass_kernel_spmd(
                nc, in_maps, core_ids=list(range(E))
            )
            break
        except Exception as err:  # rare transient NRT device errors
            last_err = err
    if res is None:
        raise last_err
    LAST_RESULT = res

    # Unshard: y_e[t, h*128+p] = sum_s yp[s, h, p, t]
    def combine_w(idx, e):
        sel = topk_idx[idx] == e  # [n, TOP_K]
        return (topk_w[idx] * sel).sum(axis=1).astype(np.float32)

    def silu(v):
        return v / (1.0 + np.exp(-v))

    out = np.zeros((T, H), np.float32)
    for e in range(E):
        ype = res.results[e]["yp"]  # [2, NH, nblk, 128, BLK] fp32
        # y_e[b*BLK+t', h*128+p] = sum_s ype[s, h, b, p, t']
        y_t = (ype[0] + ype[1]).transpose(1, 3, 0, 2).reshape(c_pad, H)
        idx_dev = expert_tokens[e][:c_pad]
        out[idx_dev] += combine_w(idx_dev, e)[:, None] * y_t[: idx_dev.size]
        # capacity spill: host fp32 pass for the few overflow tokens
        idx_sp = expert_tokens[e][c_pad:]
        if idx_sp.size:
            x_sp = xf[idx_sp]
            y_sp = (silu(x_sp @ wg[e].T) * (x_sp @ wu[e].T)) @ wd[e].T
            out[idx_sp] += combine_w(idx_sp, e)[:, None] * y_sp

    aux = _aux_loss(probs, topk_idx, T)
    return out.reshape(B, S, H), aux
